# revision 1
# baseline (speedup 1.0000x reference)
"""Single-head attention (B=8, S=2048, D=U=1024) on 8 TRN2 NeuronCores.

Sharding: data-parallel over batch — core b computes batch b end-to-end,
no cross-core communication.

Per-core pipeline (all matmuls bf16, fp32 PSUM accumulation):
  A. x [S,D] f32 --SWDGE cast--> DRAM bf16 staging blocks --xbar DMA
     transpose (sync HWDGE ring)--> xT [D,S] in SBUF.  The DRAM bounce
     exists because large xbar transposes need a DRAM source.
  B. W* f32 --SWDGE cast--> SBUF bf16 (half-width tiles, double buffered).
     SWDGE queue order (= emission order) is Wq.0, x blocks 0-3, Wq.1,
     Wk.0, Wk.1, Wv.0, Wv.1 — each arrives just before its consumer.
  C. Qt = (Wq^T xT + bq)/32  [U,S]   (lhsT=Wq, rhs=xT; bias+scale in epilogue)
     Kt = Wk^T xT + bk       [U,S]
     V  = xT^T Wv + bv       [S,U]   (lhsT=xT, rhs=Wv; bv broadcast-added in
     the DVE epilogue)
  D. scores^T[k,q] = sum_u Kt[u,k] Qt[u,q]; the padding mask adds the rank-1
     term c_k*m_q (c = -10000*(1-m)) via one DVE scalar_tensor_tensor per
     PSUM tile; Et = exp(scores^T) on ACT, PSUM->SBUF bf16.  No
     max-subtraction: scores are O(1) and masked entries underflow to
     exactly 0, matching the fp32 reference.
  E. ctx[q,u] = sum_k Et[k,q]^T V[k,u]  (lhsT=Et -> natural output layout);
     denom[q] via extra N=1 ones-column matmul under the same stationary Et;
     out = ctx * (1/denom) in the PSUM->SBUF epilogue (per-partition scalar).

SBUF: one long-lived pool; xT (phases A-C) and Et (D-E) share a 64KB tag
slot; small staging tiles and the E-phase output/reciprocal tiles reuse the
qt/kt/v tag slots outside those tensors' live ranges.
"""

import os
import sys

import numpy as np

for _p in ("/opt/trn_rl_repo", "/opt/pypackages"):
    if _p not in sys.path and os.path.isdir(_p):
        sys.path.append(_p)

import concourse.bass as bass
import concourse.tile as tile
from concourse import bacc, mybir
from concourse.bass import ts
from concourse.bass_utils import run_bass_kernel_spmd

P = 128
B, S, D, U = 8, 2048, 1024, 1024
NCORES = 8
NG = 512  # matmul moving free dim (one fp32 PSUM bank)
DT, UT, ST, KT = D // P, U // P, S // P, S // P  # 8, 8, 16, 16
SG, QG = S // NG, S // NG  # 4, 4
UG = U // NG  # 2
UH = UT // 2  # u-tiles per W half
SCALE = 1.0 / 32.0  # 1/sqrt(U)

F32 = mybir.dt.float32
BF16 = mybir.dt.bfloat16
I32 = mybir.dt.int32
AF = mybir.ActivationFunctionType
ALU = mybir.AluOpType

_cache = {}
last_results = None


def _emit(tc):
    nc = tc.nc
    x_d = nc.dram_tensor("x", [S, D], F32, kind="ExternalInput").ap()
    m_d = nc.dram_tensor("mask", [1, S], I32, kind="ExternalInput").ap()
    w_d = {
        "q": nc.dram_tensor("wq", [D, U], F32, kind="ExternalInput").ap(),
        "k": nc.dram_tensor("wk", [D, U], F32, kind="ExternalInput").ap(),
        "v": nc.dram_tensor("wv", [D, U], F32, kind="ExternalInput").ap(),
    }
    bq_d = nc.dram_tensor("bq", [1, U], F32, kind="ExternalInput").ap()
    bk_d = nc.dram_tensor("bk", [1, U], F32, kind="ExternalInput").ap()
    bv_d = nc.dram_tensor("bv", [1, U], F32, kind="ExternalInput").ap()
    out_d = nc.dram_tensor("out", [S, U], F32, kind="ExternalOutput").ap()

    # ---------------- small persistent tensors ----------------
    consts, free_consts = tc.tile(shape=[P, 2 * UT + KT], dtype=F32, name="consts")
    bq_cols = consts[:, 0:UT]
    bk_cols = consts[:, UT : 2 * UT]
    c_cols = consts[:, 2 * UT : 2 * UT + KT]  # -10000*(1-m), per k partition

    rows, free_rows = tc.tile(shape=[1, S + U + P], dtype=BF16, name="rows")
    m_row = rows[:, 0:S]
    bv_row = rows[:, S : S + U]
    ones_row = rows[:, S + U : S + U + P]

    ones_col, free_ones_col = tc.tile(shape=[P, 1], dtype=BF16, name="ones_col")
    m_bcast, free_m_bcast = tc.tile(shape=[P, S], dtype=BF16, name="m_bcast")
    bv_bcast, free_bv_bcast = tc.tile(shape=[P, U], dtype=BF16, name="bv_bcast")

    with tc.tile_pool(name="big", bufs=1) as big:

        def load_w_half(which, half):
            wt = big.tile([P, DT, NG], BF16, tag="w", bufs=2, name=f"w{which}_{half}")
            src = w_d[which].rearrange("(t p) u -> p t u", p=P)[:, :, ts(half, NG)]
            nc.gpsimd.dma_start(wt[:], src)  # f32 -> bf16 cast (SWDGE)
            return wt

        wq_h = [load_w_half("q", 0)]

        # small HWDGE loads up front (a few KB; must not trail the 32
        # transposes in the HWDGE queue)
        nc.sync.dma_start(bq_cols, bq_d.rearrange("a (j p) -> p (a j)", p=P))
        nc.sync.dma_start(bk_cols, bk_d.rearrange("a (j p) -> p (a j)", p=P))
        nc.vector.memset(ones_row, 1.0)
        nc.vector.memset(ones_col[:], 1.0)


        # ---------------- phase A: x -> bf16 -> transpose ----------------
        # slotA holds xT (A-C) then Et (D-E); sized for Et (64KB/partition).
        # SWDGE cast-DMAs stage bf16 x in DRAM; the xbar transposes
        # (serialized ~1.26us each on the sync ring) read it back per block.
        xT = big.tile([P, DT, S], BF16, tag="slotA", name="xT")
        SB = S // SG  # 512-row staging blocks
        with tc.tile_pool(name="xstage", bufs=SG, space="DRAM") as xstage:
            for sb in range(SG):
                blk = xstage.tile([SB, D], BF16, tag="xbf", name=f"xbf_{sb}")
                nc.gpsimd.dma_start(blk[:], x_d[ts(sb, SB), :])  # f32 -> bf16
                for dt in range(DT):
                    nc.sync.dma_start_transpose(xT[:, dt, ts(sb, SB)], blk[:, ts(dt, P)])
            wq_h.append(load_w_half("q", 1))

        # staging tiles ride the qt/kt/v tag slots, which are idle until C
        m_i32 = big.tile([1, S], I32, tag="qt", name="m_i32")
        nc.sync.dma_start(m_i32[:], m_d)
        nc.vector.tensor_copy(m_row, m_i32[:])
        mk_i32 = big.tile([P, KT], I32, tag="v", name="mk_i32")
        nc.sync.dma_start(mk_i32[:], m_d.rearrange("a (t p) -> p (a t)", p=P))
        # c = m*10000 - 10000  -> 0 where m==1, -10000 where m==0
        nc.vector.tensor_scalar(
            c_cols, mk_i32[:], 10000.0, -10000.0, ALU.mult, ALU.add
        )
        bv_f32 = big.tile([1, U], F32, tag="kt", name="bv_f32")
        nc.sync.dma_start(bv_f32[:], bv_d)
        nc.vector.tensor_copy(bv_row, bv_f32[:])

        # broadcast m and bv across partitions via ones-column matmuls
        with tc.tile_pool(name="psInit", bufs=2, space="PSUM") as psInit:
            for qg in range(QG):
                pi = psInit.tile([P, NG], F32, tag="init", name="ps_init")
                nc.tensor.matmul(
                    pi[:], lhsT=ones_row[:, 0:P], rhs=m_row[:, ts(qg, NG)]
                )
                nc.vector.tensor_copy(m_bcast[:, ts(qg, NG)], pi[:])
            for ug in range(UG):
                pi = psInit.tile([P, NG], F32, tag="init", name="ps_init2")
                nc.tensor.matmul(
                    pi[:], lhsT=ones_row[:, 0:P], rhs=bv_row[:, ts(ug, NG)]
                )
                nc.vector.tensor_copy(bv_bcast[:, ts(ug, NG)], pi[:])

        # ---------------- phase C: projections ----------------
        qt_sb = big.tile([P, UT, S], BF16, tag="qt", name="qt_sb")
        kt_sb = big.tile([P, UT, S], BF16, tag="kt", name="kt_sb")
        v_sb = big.tile([P, ST, U], BF16, tag="v", name="v_sb")

        with tc.tile_pool(name="psC", bufs=8, space="PSUM") as psC:
            # Q^T and K^T: [u,s] = sum_d W[d,u] * xT[d,s]
            for which, dst, bias_cols, scale in (
                ("q", qt_sb, bq_cols, SCALE),
                ("k", kt_sb, bk_cols, None),
            ):
                for half in range(2):
                    w_h = wq_h[half] if which == "q" else load_w_half(which, half)
                    for sg in range(SG):
                        for u4 in range(UH):
                            ut = half * UH + u4
                            ps = psC.tile([P, NG], F32, tag="proj", name="ps_proj")
                            for dt in range(DT):
                                nc.tensor.matmul(
                                    ps[:],
                                    lhsT=w_h[:, dt, ts(u4, P)],
                                    rhs=xT[:, dt, ts(sg, NG)],
                                    start=(dt == 0),
                                    stop=(dt == DT - 1),
                                )
                            if scale is not None:
                                nc.vector.tensor_scalar(
                                    dst[:, ut, ts(sg, NG)],
                                    ps[:],
                                    bias_cols[:, ut : ut + 1],
                                    scale,
                                    ALU.add,
                                    ALU.mult,
                                )
                            else:
                                nc.vector.tensor_scalar_add(
                                    dst[:, ut, ts(sg, NG)],
                                    ps[:],
                                    bias_cols[:, ut : ut + 1],
                                )

            # V: [s,u] = sum_d xT[d,s] * Wv[d,u]; bv added in the epilogue
            for ug in range(UG):
                wv_h = load_w_half("v", ug)
                for st in range(ST):
                    pv = psC.tile([P, NG], F32, tag="proj", name="ps_v")
                    for dt in range(DT):
                        nc.tensor.matmul(
                            pv[:],
                            lhsT=xT[:, dt, ts(st, P)],
                            rhs=wv_h[:, dt, :],
                            start=(dt == 0),
                            stop=(dt == DT - 1),
                        )
                    nc.vector.tensor_tensor(
                        v_sb[:, st, ts(ug, NG)],
                        pv[:],
                        bv_bcast[:, ts(ug, NG)],
                        ALU.add,
                    )

        # ---------------- phase D: scores^T + mask + exp ----------------
        et_sb = big.tile([P, KT, S], BF16, tag="slotA", name="et_sb")
        with tc.tile_pool(name="psD", bufs=6, space="PSUM") as psD:
            for kt in range(KT):
                pss = [
                    psD.tile([P, NG], F32, tag="sc", name="ps_sc") for _ in range(QG)
                ]
                for ut in range(UT):
                    for qg in range(QG):
                        nc.tensor.matmul(
                            pss[qg][:],
                            lhsT=kt_sb[:, ut, ts(kt, P)],
                            rhs=qt_sb[:, ut, ts(qg, NG)],
                            start=(ut == 0),
                            stop=(ut == UT - 1),
                        )
                for qg in range(QG):
                    # scores += c_k * m_q  (rank-1 mask term, on DVE)
                    nc.vector.scalar_tensor_tensor(
                        pss[qg][:],
                        m_bcast[:, ts(qg, NG)],
                        c_cols[:, kt : kt + 1],
                        pss[qg][:],
                        ALU.mult,
                        ALU.add,
                    )
                    nc.scalar.activation(et_sb[:, kt, ts(qg, NG)], pss[qg][:], AF.Exp)

        # ---------------- phase E: PV + denom + normalize ----------------
        with (
            tc.tile_pool(name="psE", bufs=4, space="PSUM") as psE,
            tc.tile_pool(name="psDen", bufs=2, space="PSUM") as psDen,
        ):
            for qt in range(KT):
                pc = [
                    psE.tile([P, NG], F32, tag="ctx", name="ps_ctx")
                    for _ in range(UG)
                ]
                den = psDen.tile([P, 1], F32, tag="den", name="ps_den")
                for kt in range(KT):
                    lhsT = et_sb[:, kt, ts(qt, P)]
                    first, last = kt == 0, kt == KT - 1
                    for ug in range(UG):
                        nc.tensor.matmul(
                            pc[ug][:],
                            lhsT=lhsT,
                            rhs=v_sb[:, kt, ts(ug, NG)],
                            start=first,
                            stop=last,
                        )
                    nc.tensor.matmul(
                        den[:], lhsT=lhsT, rhs=ones_col[:], start=first, stop=last
                    )
                recip = big.tile([P, 1], F32, tag="kt", name="recip")
                nc.vector.reciprocal(recip[:], den[:])
                o = big.tile([P, U], F32, tag="qt", name="o_sb")
                for ug in range(UG):
                    nc.vector.tensor_scalar_mul(o[:, ts(ug, NG)], pc[ug][:], recip[:])
                nc.sync.dma_start(out_d[ts(qt, P), :], o[:])

    free_bv_bcast()
    free_m_bcast()
    free_ones_col()
    free_rows()
    free_consts()


def _build():
    if "nc" in _cache:
        return _cache["nc"]
    nc = bacc.Bacc("TRN2", target_bir_lowering=False, debug=False, num_devices=NCORES)
    with tile.TileContext(nc) as tc:
        _emit(tc)
    nc.compile()
    _cache["nc"] = nc
    return nc


def kernel(x, mask, Wq, bq, Wk, bk, Wv, bv):
    global last_results
    nc = _build()
    wq = np.ascontiguousarray(Wq, dtype=np.float32)
    wk = np.ascontiguousarray(Wk, dtype=np.float32)
    wv = np.ascontiguousarray(Wv, dtype=np.float32)
    bqr = np.ascontiguousarray(bq, dtype=np.float32).reshape(1, U)
    bkr = np.ascontiguousarray(bk, dtype=np.float32).reshape(1, U)
    bvr = np.ascontiguousarray(bv, dtype=np.float32).reshape(1, U)
    in_maps = []
    for b in range(B):
        in_maps.append(
            {
                "x": np.ascontiguousarray(x[b], dtype=np.float32),
                "mask": np.ascontiguousarray(mask[b], dtype=np.int32).reshape(1, S),
                "wq": wq,
                "wk": wk,
                "wv": wv,
                "bq": bqr,
                "bk": bkr,
                "bv": bvr,
            }
        )
    res = run_bass_kernel_spmd(
        nc,
        in_maps,
        core_ids=list(range(NCORES)),
        trace=bool(int(os.environ.get("KERNEL_TRACE", "0"))),
        tmpdir=os.environ.get("KERNEL_TRACE_DIR"),
    )
    last_results = res
    return np.stack([res.results[b]["out"] for b in range(B)])



# revision 4
# speedup vs baseline: 1.0598x; 1.0598x over previous
"""Single-head attention (B=8, S=2048, D=U=1024) on 8 TRN2 NeuronCores.

Sharding: data-parallel over batch — core b computes batch b end-to-end,
no cross-core communication.

Per-core pipeline (all matmuls bf16, fp32 PSUM accumulation):
  A. x [S,D] f32 --SWDGE cast--> SBUF bf16 staging blocks [s,d] -->
     PE-transpose (128x128 tiles through the PE array, identity rhs,
     bf16 PSUM) --> DVE/ACT copy --> xT [D,S] bf16.  No DRAM bounce and
     no serialized xbar: transposes ride the tensor queue, interleaved
     block-wise with the first Q projection groups.
  B. W* f32 --SWDGE cast--> SBUF bf16 (half-width tiles, double buffered).
     SWDGE queue order (= emission order) is x.0, Wq.0, x.1, x.2, x.3,
     Wq.1, Wk.0, Wk.1, Wv.0, Wv.1 — each arrives just before its consumer.
  C. Qt = (Wq^T xT + bq)/32  [U,S]   (lhsT=Wq, rhs=xT; bias+scale in epilogue)
     Kt = Wk^T xT + bk       [U,S]
     V  = xT^T Wv + bv       [S,U]   (lhsT=xT, rhs=Wv; bv broadcast-added in
     the DVE epilogue)
  D. scores^T[k,q] = sum_u Kt[u,k] Qt[u,q]; the padding mask adds the rank-1
     term c_k*m_q (c = -10000*(1-m)) via one DVE scalar_tensor_tensor per
     PSUM tile; Et = exp(scores^T) on ACT, PSUM->SBUF bf16.  No
     max-subtraction: scores are O(1) and masked entries underflow to
     exactly 0, matching the fp32 reference.
  E. ctx[q,u] = sum_k Et[k,q]^T V[k,u]  (lhsT=Et -> natural output layout);
     denom[q] via extra N=1 ones-column matmul under the same stationary Et;
     out = ctx * (1/denom) in the PSUM->SBUF epilogue, stored and DMA'd
     as bf16 (host upcasts to f32).
"""

import os
import sys

import numpy as np

for _p in ("/opt/trn_rl_repo", "/opt/pypackages"):
    if _p not in sys.path and os.path.isdir(_p):
        sys.path.append(_p)

import concourse.bass as bass
import concourse.tile as tile
from concourse import bacc, mybir
from concourse.bass import ts
from concourse.bass_utils import run_bass_kernel_spmd
from concourse.masks import make_identity

P = 128
B, S, D, U = 8, 2048, 1024, 1024
NCORES = 8
NG = 512  # matmul moving free dim (one fp32 PSUM bank)
DT, UT, ST, KT = D // P, U // P, S // P, S // P  # 8, 8, 16, 16
SG, QG = S // NG, S // NG  # 4, 4
UG = U // NG  # 2
UH = UT // 2  # u-tiles per W half
SCALE = 1.0 / 32.0  # 1/sqrt(U)

F32 = mybir.dt.float32
BF16 = mybir.dt.bfloat16
I32 = mybir.dt.int32
AF = mybir.ActivationFunctionType
ALU = mybir.AluOpType

_cache = {}
last_results = None


def _emit(tc):
    nc = tc.nc
    x_d = nc.dram_tensor("x", [S, D], F32, kind="ExternalInput").ap()
    m_d = nc.dram_tensor("mask", [1, S], I32, kind="ExternalInput").ap()
    w_d = {
        "q": nc.dram_tensor("wq", [D, U], F32, kind="ExternalInput").ap(),
        "k": nc.dram_tensor("wk", [D, U], F32, kind="ExternalInput").ap(),
        "v": nc.dram_tensor("wv", [D, U], F32, kind="ExternalInput").ap(),
    }
    bq_d = nc.dram_tensor("bq", [1, U], F32, kind="ExternalInput").ap()
    bk_d = nc.dram_tensor("bk", [1, U], F32, kind="ExternalInput").ap()
    bv_d = nc.dram_tensor("bv", [1, U], F32, kind="ExternalInput").ap()
    out_d = nc.dram_tensor("out", [S, U], BF16, kind="ExternalOutput").ap()

    # ---------------- small persistent tensors ----------------
    consts, free_consts = tc.tile(shape=[P, 2 * UT + KT], dtype=F32, name="consts")
    bq_cols = consts[:, 0:UT]
    bk_cols = consts[:, UT : 2 * UT]
    c_cols = consts[:, 2 * UT : 2 * UT + KT]  # -10000*(1-m), per k partition

    rows, free_rows = tc.tile(shape=[1, S + U + P], dtype=BF16, name="rows")
    m_row = rows[:, 0:S]
    bv_row = rows[:, S : S + U]
    ones_row = rows[:, S + U : S + U + P]

    ones_col, free_ones_col = tc.tile(shape=[P, 1], dtype=BF16, name="ones_col")
    m_bcast, free_m_bcast = tc.tile(shape=[P, S], dtype=BF16, name="m_bcast")
    bv_bcast, free_bv_bcast = tc.tile(shape=[P, U], dtype=BF16, name="bv_bcast")
    ident, free_ident = tc.tile(shape=[P, P], dtype=BF16, name="ident")

    with tc.tile_pool(name="big", bufs=1) as big:

        def load_w_half(which, half):
            wt = big.tile([P, DT, NG], BF16, tag="w", bufs=2, name=f"w{which}_{half}")
            src = w_d[which].rearrange("(t p) u -> p t u", p=P)[:, :, ts(half, NG)]
            nc.gpsimd.dma_start(wt[:], src)  # f32 -> bf16 cast (SWDGE)
            return wt

        # identity for PE transposes; must precede W loads in the gpsimd queue
        make_identity(nc, ident)

        # x staging blocks on SWDGE, interleaved with the W halves so each
        # arrives just before its consumer.  First block + Wq.0 gate the
        # first Q group.
        xblk = []

        def load_x_block(sb):
            stg = big.tile([P, 4, D], BF16, tag="stg", bufs=2, name=f"stg_{sb}")
            nc.gpsimd.dma_start(
                stg[:], x_d[ts(sb, NG), :].rearrange("(t p) d -> p t d", p=P)
            )
            return stg

        xblk.append(load_x_block(0))
        wq_h = [load_w_half("q", 0)]
        for sb in range(1, SG):
            xblk.append(load_x_block(sb))

        # small HWDGE loads on the (otherwise idle) sync queue
        nc.sync.dma_start(bq_cols, bq_d.rearrange("a (j p) -> p (a j)", p=P))
        nc.sync.dma_start(bk_cols, bk_d.rearrange("a (j p) -> p (a j)", p=P))
        nc.vector.memset(ones_row, 1.0)
        nc.vector.memset(ones_col[:], 1.0)

        # staging tiles ride the qt/kt/v tag slots, which are idle until C
        m_i32 = big.tile([1, S], I32, tag="qt", name="m_i32")
        nc.sync.dma_start(m_i32[:], m_d)
        nc.vector.tensor_copy(m_row, m_i32[:])
        mk_i32 = big.tile([P, KT], I32, tag="v", name="mk_i32")
        nc.sync.dma_start(mk_i32[:], m_d.rearrange("a (t p) -> p (a t)", p=P))
        # c = m*10000 - 10000  -> 0 where m==1, -10000 where m==0
        nc.vector.tensor_scalar(
            c_cols, mk_i32[:], 10000.0, -10000.0, ALU.mult, ALU.add
        )
        bv_f32 = big.tile([1, U], F32, tag="kt", name="bv_f32")
        nc.sync.dma_start(bv_f32[:], bv_d)
        nc.vector.tensor_copy(bv_row, bv_f32[:])

        # broadcast m and bv across partitions via ones-column matmuls
        with tc.tile_pool(name="psInit", bufs=2, space="PSUM") as psInit:
            for qg in range(QG):
                pi = psInit.tile([P, NG], F32, tag="init", name="ps_init")
                nc.tensor.matmul(
                    pi[:], lhsT=ones_row[:, 0:P], rhs=m_row[:, ts(qg, NG)]
                )
                nc.vector.tensor_copy(m_bcast[:, ts(qg, NG)], pi[:])
            for ug in range(UG):
                pi = psInit.tile([P, NG], F32, tag="init", name="ps_init2")
                nc.tensor.matmul(
                    pi[:], lhsT=ones_row[:, 0:P], rhs=bv_row[:, ts(ug, NG)]
                )
                nc.vector.tensor_copy(bv_bcast[:, ts(ug, NG)], pi[:])

        # ---------------- phases A+C: transposes interleaved with Q --------
        # slotA holds xT (A-C) then Et (D-E); sized for Et (64KB/partition).
        xT = big.tile([P, DT, S], BF16, tag="slotA", name="xT")
        qt_sb = big.tile([P, UT, S], BF16, tag="qt", name="qt_sb")
        kt_sb = big.tile([P, UT, S], BF16, tag="kt", name="kt_sb")
        v_sb = big.tile([P, ST, U], BF16, tag="v", name="v_sb")

        with (
            tc.tile_pool(name="psA", bufs=2, space="PSUM") as psA,
            tc.tile_pool(name="psC", bufs=6, space="PSUM") as psC,
        ):

            def proj_group(dst, bias_cols, scale, sg, half, w_h):
                for u4 in range(UH):
                    ut = half * UH + u4
                    ps = psC.tile([P, NG], F32, tag="proj", name="ps_proj")
                    for dt in range(DT):
                        nc.tensor.matmul(
                            ps[:],
                            lhsT=w_h[:, dt, ts(u4, P)],
                            rhs=xT[:, dt, ts(sg, NG)],
                            start=(dt == 0),
                            stop=(dt == DT - 1),
                        )
                    if scale is not None:
                        nc.vector.tensor_scalar(
                            dst[:, ut, ts(sg, NG)],
                            ps[:],
                            bias_cols[:, ut : ut + 1],
                            scale,
                            ALU.add,
                            ALU.mult,
                        )
                    else:
                        nc.vector.tensor_scalar_add(
                            dst[:, ut, ts(sg, NG)], ps[:], bias_cols[:, ut : ut + 1]
                        )

            for sb in range(SG):
                # transpose block sb: 32 PE tile-transposes, drained per dt
                for dt in range(DT):
                    pt = psA.tile([P, NG], BF16, tag="tp", name="ps_tp")
                    for t in range(4):
                        nc.tensor.transpose(
                            pt[:, ts(t, P)], xblk[sb][:, t, ts(dt, P)], ident[:]
                        )
                    if dt % 2 == 0:
                        nc.vector.tensor_copy(xT[:, dt, ts(sb, NG)], pt[:])
                    else:
                        nc.scalar.copy(xT[:, dt, ts(sb, NG)], pt[:])
                # overlap: Q projection for this block's s-range, half 0
                proj_group(qt_sb, bq_cols, SCALE, sb, 0, wq_h[0])
            wq_h.append(load_w_half("q", 1))
            for sg in range(SG):
                proj_group(qt_sb, bq_cols, SCALE, sg, 1, wq_h[1])

            # K^T
            for half in range(2):
                wk_h = load_w_half("k", half)
                for sg in range(SG):
                    proj_group(kt_sb, bk_cols, None, sg, half, wk_h)

            # V: [s,u] = sum_d xT[d,s] * Wv[d,u]; bv added in the epilogue
            for ug in range(UG):
                wv_h = load_w_half("v", ug)
                for st in range(ST):
                    pv = psC.tile([P, NG], F32, tag="proj", name="ps_v")
                    for dt in range(DT):
                        nc.tensor.matmul(
                            pv[:],
                            lhsT=xT[:, dt, ts(st, P)],
                            rhs=wv_h[:, dt, :],
                            start=(dt == 0),
                            stop=(dt == DT - 1),
                        )
                    nc.vector.tensor_tensor(
                        v_sb[:, st, ts(ug, NG)],
                        pv[:],
                        bv_bcast[:, ts(ug, NG)],
                        ALU.add,
                    )

        # ---------------- phase D: scores^T + mask + exp ----------------
        et_sb = big.tile([P, KT, S], BF16, tag="slotA", name="et_sb")
        with tc.tile_pool(name="psD", bufs=6, space="PSUM") as psD:
            for kt in range(KT):
                pss = [
                    psD.tile([P, NG], F32, tag="sc", name="ps_sc") for _ in range(QG)
                ]
                for ut in range(UT):
                    for qg in range(QG):
                        nc.tensor.matmul(
                            pss[qg][:],
                            lhsT=kt_sb[:, ut, ts(kt, P)],
                            rhs=qt_sb[:, ut, ts(qg, NG)],
                            start=(ut == 0),
                            stop=(ut == UT - 1),
                        )
                for qg in range(QG):
                    # scores += c_k * m_q  (rank-1 mask term, on DVE)
                    nc.vector.scalar_tensor_tensor(
                        pss[qg][:],
                        m_bcast[:, ts(qg, NG)],
                        c_cols[:, kt : kt + 1],
                        pss[qg][:],
                        ALU.mult,
                        ALU.add,
                    )
                    nc.scalar.activation(et_sb[:, kt, ts(qg, NG)], pss[qg][:], AF.Exp)

        # ---------------- phase E: PV + denom + normalize ----------------
        with (
            tc.tile_pool(name="psE", bufs=4, space="PSUM") as psE,
            tc.tile_pool(name="psDen", bufs=2, space="PSUM") as psDen,
        ):
            for qt in range(KT):
                pc = [
                    psE.tile([P, NG], F32, tag="ctx", name="ps_ctx")
                    for _ in range(UG)
                ]
                den = psDen.tile([P, 1], F32, tag="den", name="ps_den")
                for kt in range(KT):
                    lhsT = et_sb[:, kt, ts(qt, P)]
                    first, last = kt == 0, kt == KT - 1
                    for ug in range(UG):
                        nc.tensor.matmul(
                            pc[ug][:],
                            lhsT=lhsT,
                            rhs=v_sb[:, kt, ts(ug, NG)],
                            start=first,
                            stop=last,
                        )
                    nc.tensor.matmul(
                        den[:], lhsT=lhsT, rhs=ones_col[:], start=first, stop=last
                    )
                recip = big.tile([P, 1], F32, tag="kt", name="recip")
                nc.vector.reciprocal(recip[:], den[:])
                o = big.tile([P, U], BF16, tag="qt", name="o_sb")
                for ug in range(UG):
                    nc.vector.tensor_scalar_mul(o[:, ts(ug, NG)], pc[ug][:], recip[:])
                nc.sync.dma_start(out_d[ts(qt, P), :], o[:])

    free_ident()
    free_bv_bcast()
    free_m_bcast()
    free_ones_col()
    free_rows()
    free_consts()


def _build():
    if "nc" in _cache:
        return _cache["nc"]
    nc = bacc.Bacc("TRN2", target_bir_lowering=False, debug=False, num_devices=NCORES)
    with tile.TileContext(nc) as tc:
        _emit(tc)
    nc.compile()
    _cache["nc"] = nc
    return nc


def kernel(x, mask, Wq, bq, Wk, bk, Wv, bv):
    global last_results
    nc = _build()
    wq = np.ascontiguousarray(Wq, dtype=np.float32)
    wk = np.ascontiguousarray(Wk, dtype=np.float32)
    wv = np.ascontiguousarray(Wv, dtype=np.float32)
    bqr = np.ascontiguousarray(bq, dtype=np.float32).reshape(1, U)
    bkr = np.ascontiguousarray(bk, dtype=np.float32).reshape(1, U)
    bvr = np.ascontiguousarray(bv, dtype=np.float32).reshape(1, U)
    in_maps = []
    for b in range(B):
        in_maps.append(
            {
                "x": np.ascontiguousarray(x[b], dtype=np.float32),
                "mask": np.ascontiguousarray(mask[b], dtype=np.int32).reshape(1, S),
                "wq": wq,
                "wk": wk,
                "wv": wv,
                "bq": bqr,
                "bk": bkr,
                "bv": bvr,
            }
        )
    res = run_bass_kernel_spmd(
        nc,
        in_maps,
        core_ids=list(range(NCORES)),
        trace=bool(int(os.environ.get("KERNEL_TRACE", "0"))),
        tmpdir=os.environ.get("KERNEL_TRACE_DIR"),
    )
    last_results = res
    return np.stack([res.results[b]["out"].astype(np.float32) for b in range(B)])
